# revision 1
# baseline (speedup 1.0000x reference)
"""Trainium2 Bass kernel for nn_MultiHeadAttention_48086453846410.

Reference computation (heads folded into the sequence axis, softmax over the
FULL L = seq*heads key axis):
    qp = (q @ wk_w.T + wk_b).reshape(bs, L, d)   # note swapped wk/wq, faithful
    kp = (k @ wq_w.T + wq_b).reshape(bs, L, d)
    vp = (v @ wv_w.T + wv_b).reshape(bs, L, d)
    scores = qp @ kp.T / sqrt(d); attn = softmax(scores, -1)
    o = (attn @ vp).reshape(bs, seq, d*heads)
    out = o @ out_w.T + out_b

Sharding: 8 cores = (batch b in 0..3) x (seq half). Each core owns 256 query
seq positions of one batch (2048 query rows l' = h*256+s). Softmax is over
keys, so query rows are independent -> no collectives.

On-device layout strategy (all matmuls bf16 inputs, fp32 PSUM accumulate):
 - host pre-transposes activations/weights so no on-device transposes at all
 - qpT (interleaved d-tile-major layout so score matmuls take two heads per
   N=512 moving operand) / kpT computed transposed (proj dim j on partitions)
 - vp computed in natural layout (t on partitions)
 - scores computed transposed: scoresT[m=(g,t), l'] -> softmax needs only
   exp (scores bounded: |s| < 2, so no max subtraction) and the denominator
   Z. The chunk dimension of the Z reduction is elementwise over (m, l), so
   it runs as a chained fp32 accumulate on the otherwise-idle gpsimd engine;
   only the final 128-partition reduction uses the PE (one fp32 ones-matmul
   per l-slice, replicated across partitions for free). Normalization is
   deferred to oT columns.
 - attn@v consumes exp tiles directly as the moving operand -> oT (e on
   partitions), which is exactly the lhsT layout for the out projection.
 - out projection runs per l-slice (pipelined into phase B) with fp32
   partial sums held in SBUF via DVE adds.
 - the PE queue is strict FIFO, so accumulation matmuls that wait on the
   previous slice's PSUM drain are emission-deferred (Z/attn by a 3-chunk
   skew, each slice's epilogue into the next slice's chunk stream) to keep
   independent scores work ahead of them.

Rejected after measurement: sharing the duplicated kp/vp projections across
the core pair of each batch via pairwise AllGather — a chained-AllGather
microbenchmark on this hardware measured 200-350us per 2MB collective
(vs 27us of PE saved), so the duplication is cheaper.
"""

import math
import sys

for _p in ("/opt/trn_rl_repo",):
    if _p not in sys.path:
        sys.path.insert(0, _p)

import numpy as np
import ml_dtypes

BS, SEQ, D, HEADS = 4, 512, 512, 8
NCORES = 8
S = SEQ // 2            # 256 query seq rows per core
JT = HEADS * D // 128   # 32 tiles of the 4096 projection dim
DT = D // 128           # 4 tiles of the 512 contraction dim
TT = SEQ // 128         # 4 key-seq tiles per head
LSLICES = 4             # l' = 2048 per core, processed in 4 slices of 512
WQCOLS = 1024           # weight streaming tile width (quarter tiles)
NP_BF16 = ml_dtypes.bfloat16

_CACHE = {}


def _build_program():
    from concourse import bacc
    import concourse.mybir as mybir
    import concourse.tile as tile
    from concourse.dt import dt

    f32 = dt.float32
    b16 = dt.bfloat16
    Act = mybir.ActivationFunctionType

    nc = bacc.Bacc(None, target_bir_lowering=False, debug=False,
                   num_devices=NCORES)

    def din(name, shape, dty=b16):
        return nc.dram_tensor(name, shape, dty, kind="ExternalInput").ap()

    qT = din("qT", [D, S])                 # q[b, half].T      (d, s)
    kT = din("kT", [D, SEQ])               # k[b].T            (d, t)
    vT = din("vT", [D, SEQ])               # v[b].T            (d, t)
    wkT = din("wkT", [D, HEADS * D])       # wk_w.T            (d, j)
    wqT = din("wqT", [D, HEADS * D])       # wq_w.T            (d, j)
    wvT = din("wvT", [D, HEADS * D])       # wv_w.T            (d, j)
    owT = din("owT", [HEADS * D, D])       # out_w.T           (c, r)
    wk_bT = din("wk_bT", [128, JT], f32)   # wk_b.reshape(JT,128).T
    wq_bT = din("wq_bT", [128, JT], f32)
    wv_br = din("wv_br", [128, HEADS * D], f32)   # wv_b replicated
    out_br = din("out_br", [128, D], f32)         # out_b replicated
    ones = din("ones", [128, 128])
    out = nc.dram_tensor("out", [S, D], f32, kind="ExternalOutput").ap()

    inv_sqrt_d = 1.0 / math.sqrt(D)
    NWQ = (HEADS * D) // WQCOLS  # 4 quarter-tiles per d-tile row

    with tile.TileContext(nc) as tc:
        with (
            tc.tile_pool(name="const", bufs=1) as cp,
            tc.tile_pool(name="wpool", bufs=20) as wp,
            tc.tile_pool(name="acts", bufs=1) as acp,
            tc.tile_pool(name="state", bufs=1) as sp,
            tc.tile_pool(name="expp", bufs=8) as ep,
            tc.tile_pool(name="zrp", bufs=2) as zp,
            tc.tile_pool(name="owp", bufs=8) as owp,
            tc.tile_pool(name="psA", bufs=4, space="PSUM") as psA,
            tc.tile_pool(name="psO", bufs=4, space="PSUM") as psO,
            tc.tile_pool(name="zaccp", bufs=2) as zaccp,
        ):
            # ---- weight streaming: quarter tiles (128 x WQCOLS) ----
            # tile index (dt, wq) covers d rows [dt*128,...), j cols
            # [wq*WQCOLS,...). Emission order = consumption order.
            def load_w(dram, nm, engines=None):
                engines = engines or [nc.sync, nc.gpsimd]
                tiles = {}
                i = 0
                for wq in range(NWQ):
                    for dt_ in range(DT):
                        t = wp.tile([128, WQCOLS], b16, tag="w",
                                    name=f"w_{nm}_{dt_}_{wq}")
                        engines[i % len(engines)].dma_start(
                            out=t,
                            in_=dram[dt_ * 128:(dt_ + 1) * 128,
                                     wq * WQCOLS:(wq + 1) * WQCOLS])
                        i += 1
                        tiles[(dt_, wq)] = t
                return tiles

            def wslice(tiles, dt_, j0, width):
                wq, off = divmod(j0, WQCOLS)
                assert off + width <= WQCOLS
                return tiles[(dt_, wq)][:, off:off + width]

            # phase-A1 critical path first: qT (small) then wk weights
            qT_sb = acp.tile([128, DT * S], b16, tag="qT")
            nc.gpsimd.dma_start(out=qT_sb.rearrange("p (t n) -> p t n", n=S),
                                in_=qT.rearrange("(t p) n -> p t n", p=128))
            wk_bT_sb = cp.tile([128, JT], f32, tag="wkb")
            nc.sync.dma_start(out=wk_bT_sb, in_=wk_bT)
            wk_sb = load_w(wkT, "k", engines=[nc.sync, nc.gpsimd, nc.scalar])

            kT_sb = acp.tile([128, DT * SEQ], b16, tag="kT")
            vT_sb = acp.tile([128, DT * SEQ], b16, tag="vT")
            nc.sync.dma_start(out=kT_sb.rearrange("p (t n) -> p t n", n=SEQ),
                              in_=kT.rearrange("(t p) n -> p t n", p=128))
            wq_bT_sb = cp.tile([128, JT], f32, tag="wqb")
            nc.sync.dma_start(out=wq_bT_sb, in_=wq_bT)

            # ---- persistent state ----
            # qpT interleaved: col block (dt*HEADS + h)*S
            qpT_sb = sp.tile([128, JT * S], b16, tag="qpT")       # 16KB/part
            kpT_sb = sp.tile([128, JT * SEQ], b16, tag="kpT")     # 32KB/part
            vp_sb = sp.tile([128, TT * HEADS * D], b16, tag="vp")  # 32KB/part
            oT_sb = sp.tile([128, DT * 2048], b16, tag="oT")      # 16KB/part
            fin32 = sp.tile([128, 2 * D], f32, tag="fin32")       # 4KB/part

            # ---- phase A1: qpT[j, s] = wkT.T @ qT + wk_b ----
            for jt in range(JT):
                h, dt_of_j = divmod(jt, DT)
                ps = psA.tile([128, 512], f32, tag="psA")
                for dt_ in range(DT):
                    nc.tensor.matmul(
                        ps[:, :S],
                        lhsT=wslice(wk_sb, dt_, jt * 128, 128),
                        rhs=qT_sb[:, dt_ * S:(dt_ + 1) * S],
                        start=(dt_ == 0), stop=(dt_ == DT - 1))
                nc.scalar.activation(
                    qpT_sb[:, (dt_of_j * HEADS + h) * S:
                           (dt_of_j * HEADS + h + 1) * S],
                    ps[:, :S], Act.Identity,
                    bias=wk_bT_sb[:, jt:jt + 1], scale=1.0)

            # ---- phase A2: kpT[j, t] = wqT.T @ kT + wq_b ----
            wq_sb = load_w(wqT, "q")
            nc.gpsimd.dma_start(out=vT_sb.rearrange("p (t n) -> p t n", n=SEQ),
                                in_=vT.rearrange("(t p) n -> p t n", p=128))
            for jt in range(JT):
                ps = psA.tile([128, 512], f32, tag="psA")
                for dt_ in range(DT):
                    nc.tensor.matmul(
                        ps,
                        lhsT=wslice(wq_sb, dt_, jt * 128, 128),
                        rhs=kT_sb[:, dt_ * SEQ:(dt_ + 1) * SEQ],
                        start=(dt_ == 0), stop=(dt_ == DT - 1))
                nc.scalar.activation(kpT_sb[:, jt * SEQ:(jt + 1) * SEQ], ps,
                                     Act.Identity, bias=wq_bT_sb[:, jt:jt + 1],
                                     scale=1.0)

            # ---- phase A3: vp[t, j] = vT.T @ wvT + wv_b (natural layout) ----
            wv_sb = load_w(wvT, "v")
            wv_br_sb = cp.tile([128, HEADS * D], f32, tag="wvb")
            nc.sync.dma_start(out=wv_br_sb, in_=wv_br)
            ones_sb = cp.tile([128, 128], b16, tag="ones")
            nc.sync.dma_start(out=ones_sb, in_=ones)
            out_br_sb = cp.tile([128, D], f32, tag="outb")
            nc.sync.dma_start(out=out_br_sb, in_=out_br)
            for tt in range(TT):
                for js in range(HEADS):
                    ps = psA.tile([128, 512], f32, tag="psA")
                    for dt_ in range(DT):
                        nc.tensor.matmul(
                            ps,
                            lhsT=vT_sb[:, dt_ * SEQ + tt * 128:
                                       dt_ * SEQ + (tt + 1) * 128],
                            rhs=wslice(wv_sb, dt_, js * 512, 512),
                            start=(dt_ == 0), stop=(dt_ == DT - 1))
                    nc.vector.tensor_add(
                        vp_sb[:, tt * HEADS * D + js * 512:
                              tt * HEADS * D + (js + 1) * 512],
                        ps, wv_br_sb[:, js * 512:(js + 1) * 512])

            # ---- phase B + pipelined out-projection, 4 l-slices ----
            prev_outproj = [None]

            for ls in range(LSLICES):
                h0 = 2 * ls
                zacc = zaccp.tile([128, 512], f32, tag="zacc",
                                  name=f"zacc{ls}")
                po = [psO.tile([128, 512], f32, tag="psO", name=f"po{ls}_{i}")
                      for i in range(DT)]
                nchunk = HEADS * TT  # 32
                # Z/attn accumulation MMs are emitted SKEW chunks behind the
                # scores+exp of their chunk: at a slice start they block on
                # the previous slice's PSUM drain, and the PE queue is strict
                # FIFO - the skew puts independent scores work ahead of them.
                SKEW = 3
                pending = []

                def emit_zattn(ci, g, tt, ex):
                    # chunk-dim reduction is elementwise over (m-part, l):
                    # accumulate on DVE in fp32; only the final 128-partition
                    # reduction needs the PE (one matmul per slice).
                    if ci == 0:
                        nc.gpsimd.tensor_copy(zacc, ex)
                    else:
                        nc.gpsimd.tensor_add(zacc, zacc, ex)
                    for et in range(DT):
                        nc.tensor.matmul(
                            po[et],
                            lhsT=vp_sb[:, tt * HEADS * D + g * 512 + et * 128:
                                       tt * HEADS * D + g * 512 + (et + 1) * 128],
                            rhs=ex,
                            start=(ci == 0), stop=(ci == nchunk - 1))

                for g in range(HEADS):
                    for tt in range(TT):
                        ci = g * TT + tt
                        ps = psA.tile([128, 512], f32, tag="psA")
                        # scoresT[(g,tt), (h0..h0+1, s)] - both heads per MM
                        for dt_ in range(DT):
                            nc.tensor.matmul(
                                ps,
                                lhsT=kpT_sb[:, (g * DT + dt_) * SEQ + tt * 128:
                                            (g * DT + dt_) * SEQ + (tt + 1) * 128],
                                rhs=qpT_sb[:, (dt_ * HEADS + h0) * S:
                                           (dt_ * HEADS + h0 + 2) * S],
                                start=(dt_ == 0), stop=(dt_ == DT - 1))
                        ex = ep.tile([128, 512], b16, tag="exp")
                        nc.scalar.activation(ex, ps, Act.Exp, bias=0.0,
                                             scale=inv_sqrt_d)
                        pending.append((ci, g, tt, ex))
                        if ci == 2 and prev_outproj[0] is not None:
                            prev_outproj[0]()
                            prev_outproj[0] = None
                        if len(pending) > SKEW:
                            emit_zattn(*pending.pop(0))
                for args in pending:
                    emit_zattn(*args)
                # Z finalization + normalization: everything is ready at
                # slice end, emit immediately (DVE overlaps the next slice's
                # first scores). Only the out-projection stays deferred.
                zacc_bf = zaccp.tile([128, 512], b16, tag="zaccb",
                                     name=f"zaccb{ls}")
                nc.gpsimd.tensor_copy(zacc_bf, zacc)
                psz = psA.tile([128, 512], f32, tag="psA", name=f"psz{ls}")
                nc.tensor.matmul(psz, lhsT=ones_sb, rhs=zacc_bf,
                                 start=True, stop=True)
                zr = zp.tile([128, 512], f32, tag="zr", name=f"zr{ls}")
                nc.vector.reciprocal(zr, psz)
                for half in range(2):
                    for et in range(DT):
                        c0 = et * 2048 + ls * 512 + half * 256
                        nc.vector.tensor_mul(
                            oT_sb[:, c0:c0 + 256],
                            po[et][:, half * 256:(half + 1) * 256],
                            zr[:, half * 256:(half + 1) * 256])

                def make_outproj(ls=ls, h0=h0):
                    def outproj():
                        # out-projection contribution of this l-slice:
                        # c-tiles ct = h*DT+et for h in (h0, h0+1)
                        ow_tiles = {}
                        for st in range(2):
                            psc = psA.tile([128, 512], f32, tag="psA",
                                           name=f"psc{ls}_{st}")
                            for ci2, ct in enumerate(
                                    range(h0 * DT, (h0 + 2) * DT)):
                                h, et = divmod(ct, DT)
                                if st == 0:
                                    ow_tiles[ct] = owp.tile(
                                        [128, D], b16, tag="ow",
                                        name=f"ow{ct}")
                                    nc.sync.dma_start(
                                        out=ow_tiles[ct],
                                        in_=owT[ct * 128:(ct + 1) * 128, :])
                                nc.tensor.matmul(
                                    psc,
                                    lhsT=oT_sb[:, et * 2048 + h * S + st * 128:
                                               et * 2048 + h * S +
                                               (st + 1) * 128],
                                    rhs=ow_tiles[ct],
                                    start=(ci2 == 0),
                                    stop=(ci2 == 2 * DT - 1))
                            if ls == 0:
                                nc.vector.tensor_add(
                                    fin32[:, st * D:(st + 1) * D],
                                    psc, out_br_sb)
                            else:
                                nc.vector.tensor_add(
                                    fin32[:, st * D:(st + 1) * D],
                                    psc, fin32[:, st * D:(st + 1) * D])
                    return outproj

                prev_outproj[0] = make_outproj()

            prev_outproj[0]()
            for st in range(2):
                nc.sync.dma_start(out=out[st * 128:(st + 1) * 128, :],
                                  in_=fin32[:, st * D:(st + 1) * D])

    nc.compile()
    return nc


def _get_program():
    if "nc" not in _CACHE:
        _CACHE["nc"] = _build_program()
    return _CACHE["nc"]


def _prep_shared(inputs):
    bf = NP_BF16
    f32c = np.ascontiguousarray
    shared = {
        "wkT": f32c(np.asarray(inputs["wk_w"], np.float32).T).astype(bf),
        "wqT": f32c(np.asarray(inputs["wq_w"], np.float32).T).astype(bf),
        "wvT": f32c(np.asarray(inputs["wv_w"], np.float32).T).astype(bf),
        "owT": f32c(np.asarray(inputs["out_w"], np.float32).T).astype(bf),
        "wk_bT": f32c(np.asarray(inputs["wk_b"], np.float32).reshape(JT, 128).T),
        "wq_bT": f32c(np.asarray(inputs["wq_b"], np.float32).reshape(JT, 128).T),
        "wv_br": f32c(np.broadcast_to(
            np.asarray(inputs["wv_b"], np.float32)[None, :], (128, HEADS * D))),
        "out_br": f32c(np.broadcast_to(
            np.asarray(inputs["out_b"], np.float32)[None, :], (128, D))),
        "ones": np.ones((128, 128), bf),
    }
    return shared


def _make_in_maps(inputs):
    bf = NP_BF16
    shared = _prep_shared(inputs)
    q = np.asarray(inputs["q"], np.float32)
    k = np.asarray(inputs["k"], np.float32)
    v = np.asarray(inputs["v"], np.float32)
    in_maps = []
    for core in range(NCORES):
        b, half = divmod(core, 2)
        m = dict(shared)
        m["qT"] = np.ascontiguousarray(q[b, half * S:(half + 1) * S, :].T).astype(bf)
        m["kT"] = np.ascontiguousarray(k[b].T).astype(bf)
        m["vT"] = np.ascontiguousarray(v[b].T).astype(bf)
        in_maps.append(m)
    return in_maps


def kernel(**inputs):
    from concourse.bass_utils import run_bass_kernel_spmd

    nc = _get_program()
    in_maps = _make_in_maps(inputs)
    res = run_bass_kernel_spmd(nc, in_maps, core_ids=list(range(NCORES)))
    _CACHE["last_results"] = res
    out = np.empty((BS, SEQ, D), np.float32)
    for core in range(NCORES):
        b, half = divmod(core, 2)
        out[b, half * S:(half + 1) * S, :] = res.results[core]["out"]
    return out


if __name__ == "__main__":
    rng = np.random.default_rng(0)
    fake = {
        "q": rng.standard_normal((BS, SEQ, D)).astype(np.float32),
        "k": rng.standard_normal((BS, SEQ, D)).astype(np.float32),
        "v": rng.standard_normal((BS, SEQ, D)).astype(np.float32),
        "wq_w": (rng.standard_normal((D * HEADS, D)) * 0.02).astype(np.float32),
        "wq_b": (rng.standard_normal((D * HEADS,)) * 0.02).astype(np.float32),
        "wk_w": (rng.standard_normal((D * HEADS, D)) * 0.02).astype(np.float32),
        "wk_b": (rng.standard_normal((D * HEADS,)) * 0.02).astype(np.float32),
        "wv_w": (rng.standard_normal((D * HEADS, D)) * 0.02).astype(np.float32),
        "wv_b": (rng.standard_normal((D * HEADS,)) * 0.02).astype(np.float32),
        "out_w": (rng.standard_normal((D, D * HEADS)) * 0.02).astype(np.float32),
        "out_b": (rng.standard_normal((D,)) * 0.02).astype(np.float32),
    }
    o = kernel(**fake)
    print("kernel ran, out shape", o.shape, "std", o.std())



# revision 7
# speedup vs baseline: 1.5406x; 1.5406x over previous
"""Trainium2 Bass kernel for nn_MultiHeadAttention_48086453846410.

Reference computation (heads folded into the sequence axis, softmax over the
FULL L = seq*heads key axis):
    qp = (q @ wk_w.T + wk_b).reshape(bs, L, d)   # note swapped wk/wq, faithful
    kp = (k @ wq_w.T + wq_b).reshape(bs, L, d)
    vp = (v @ wv_w.T + wv_b).reshape(bs, L, d)
    scores = qp @ kp.T / sqrt(d); attn = softmax(scores, -1)
    o = (attn @ vp).reshape(bs, seq, d*heads)
    out = o @ out_w.T + out_b

Sharding: 8 cores = (batch b in 0..3) x (seq half). Each core owns 256 query
seq positions of one batch (2048 query rows l' = h*256+s). Softmax is over
keys, so query rows are independent -> no collectives.

Speed strategy vs the bf16 baseline (which ran at ~95% of the bf16 PE
roofline): move the q/k/scores path to fp8-e4m3 DoubleRow matmuls (cost
model: 0.5 cycles/out-row with a K=256 contraction per instruction = 4x bf16
MAC throughput), while protecting accuracy on the value path:

 - q/k projections: fp8 DR (q,k,wk,wq quantized e4m3; weights pre-scaled x64
   on host; bias+rescale+fp8-cast done in ONE gpsimd tensor_scalar per tile,
   taking the scalar engine out of the A phases entirely).
 - scores: fp8 DR over qp8/kp8 (stored x2).  Attribution runs showed the
   whole q/k path tolerates e4m3 (~5e-3 each); the v path does NOT.
 - attn@v uses the decomposition  o*Z = colsum(vp) + sum_m r_m vp8_m  with
   r = exp(s)-1 (|r| ~ 0.2): the dominant mean signal comes from an EXACT
   bf16-path colsum (one cheap [1,512] PE reduction), and fp8 noise only
   rides on the small fluctuation term -> fp8 DR for the big matmul at
   bf16-level accuracy.  Z = 4096 + sum_m r_m comes free from a DR
   ones-matmul accumulating in its own PSUM bank (replaces the baseline's
   gpsimd Z chain + final ones-matmul).  The "+4096" and "+colsum" terms are
   injected as rank-1 bf16 matmuls that START each PSUM accumulation chain.
 - v projection, colsum, and out-projection stay bf16 (v-path e4m3 was
   measured at 1.4-2.4e-2 error alone -> too close to the 2e-2 gate).
 - exp runs on the scalar engine (bf16 out); r8 = ex-1 -> fp8 on gpsimd.

Measured numpy emulation of this exact dataflow: maxrel 9.0e-3 (gate 2e-2).
PE cycle budget ~273K cycles (~114us at 2.4GHz) vs ~720K for the baseline;
phase B is jointly PE/scalar/gpsimd bound.
"""

import math
import sys

for _p in ("/opt/trn_rl_repo",):
    if _p not in sys.path:
        sys.path.insert(0, _p)

import numpy as np
import ml_dtypes

BS, SEQ, D, HEADS = 4, 512, 512, 8
NCORES = 8
S = SEQ // 2            # 256 query seq rows per core
HD = HEADS * D          # 4096 projection dim
JT = HD // 128          # 32 tiles of the projection dim
DT = D // 128           # 4 tiles of the 512 contraction dim
NP = DT // 2            # 2 DoubleRow k-tile pairs over d
TT = SEQ // 128         # 4 key-seq tiles per head
LSLICES = 4             # l' = 2048 per core, processed in 4 slices of 512
NP_BF16 = ml_dtypes.bfloat16
NP_E4 = ml_dtypes.float8_e4m3

_CACHE = {}


def _build_program():
    from concourse import bacc
    import concourse.mybir as mybir
    import concourse.tile as tile
    from concourse.dt import dt

    f32 = dt.float32
    b16 = dt.bfloat16
    f8 = dt.float8e4
    Act = mybir.ActivationFunctionType
    DR = mybir.MatmulPerfMode.DoubleRow
    ALU = mybir.AluOpType

    nc = bacc.Bacc(None, target_bir_lowering=False, debug=False,
                   num_devices=NCORES)

    def din(name, shape, dty=b16):
        return nc.dram_tensor(name, shape, dty, kind="ExternalInput").ap()

    qT8 = din("qT8", [D, S], f8)           # q[b, half].T             (d, s)
    kT8 = din("kT8", [D, SEQ], f8)         # k[b].T                   (d, t)
    vT = din("vT", [D, SEQ])               # v[b].T  bf16             (d, t)
    wk8 = din("wk8", [D, HD], f8)          # 64*wk_w.T  e4m3          (d, j)
    wq8 = din("wq8", [D, HD], f8)          # 64*wq_w.T  e4m3          (d, j)
    wvT = din("wvT", [D, HD])              # wv_w.T  bf16             (d, j)
    owT = din("owT", [HD, D])              # out_w.T  bf16            (c, r)
    wk_b2T = din("wk_b2T", [128, JT], f32)  # (2*wk_b).reshape(JT,128).T
    wq_b2T = din("wq_b2T", [128, JT], f32)
    wv_br = din("wv_br", [128, HD], f32)    # wv_b replicated
    out_br = din("out_br", [128, D], f32)   # out_b replicated
    ones8 = din("ones8", [128, 256], f8)    # 1.0s: Z DoubleRow lhsT
    consts = din("consts", [128, 640])      # [:512]=1.0  [512:640]=4096.0
    out = nc.dram_tensor("out", [S, D], f32, kind="ExternalOutput").ap()

    inv_sqrt_d = 1.0 / math.sqrt(D)

    with tile.TileContext(nc) as tc:
        with (
            tc.tile_pool(name="const", bufs=1) as cp,
            tc.tile_pool(name="wpool", bufs=16) as wp,
            tc.tile_pool(name="acts", bufs=1) as acp,
            tc.tile_pool(name="state", bufs=1) as sp,
            tc.tile_pool(name="rpairs", bufs=6) as ep,
            tc.tile_pool(name="exs", bufs=4) as xp,
            tc.tile_pool(name="zrp", bufs=2) as zp,
            tc.tile_pool(name="owp", bufs=8) as owp,
            tc.tile_pool(name="psA", bufs=3, space="PSUM") as psA,
            tc.tile_pool(name="psO", bufs=5, space="PSUM") as psO,
        ):
            # ---- fp8 weight streaming: DoubleRow pair tiles ----
            # tile (p, wq) holds d rows [p*256, (p+1)*256) as [128, 2, 1024]:
            # partition = d within 128-block, dim1 = the two d-blocks of the
            # DR pair, dim2 = j window.  Emission order = consumption order.
            def load_w8(dram_t, nm, engines):
                tiles = {}
                i = 0
                for wq_i in range(4):
                    for p in range(NP):
                        t = wp.tile([128, 2048], f8, tag="w",
                                    name=f"w8_{nm}_{p}_{wq_i}")
                        engines[i % len(engines)].dma_start(
                            out=t.rearrange("p (two j) -> p two j", two=2),
                            in_=dram_t[p * 256:(p + 1) * 256,
                                       wq_i * 1024:(wq_i + 1) * 1024]
                            .rearrange("(two p) j -> p two j", p=128))
                        tiles[(p, wq_i)] = t
                        i += 1
                return tiles

            def w8slice(tiles, p, j0):
                wq_i, off = divmod(j0, 1024)
                return tiles[(p, wq_i)].rearrange(
                    "p (two j) -> p two j", two=2)[:, :, off:off + 128]

            # bf16 quarter-tile streaming for wv (baseline scheme)
            def load_w16(dram_t, nm, engines):
                tiles = {}
                i = 0
                for wq_i in range(4):
                    for dt_ in range(DT):
                        t = wp.tile([128, 1024], b16, tag="w",
                                    name=f"w_{nm}_{dt_}_{wq_i}")
                        engines[i % len(engines)].dma_start(
                            out=t,
                            in_=dram_t[dt_ * 128:(dt_ + 1) * 128,
                                       wq_i * 1024:(wq_i + 1) * 1024])
                        tiles[(dt_, wq_i)] = t
                        i += 1
                return tiles

            def w16slice(tiles, dt_, j0, width):
                wq_i, off = divmod(j0, 1024)
                return tiles[(dt_, wq_i)][:, off:off + width]

            # phase-A1 critical path first: qT8 (small) then wk weights
            qT8_sb = acp.tile([128, DT * S], f8, tag="qT")
            nc.gpsimd.dma_start(out=qT8_sb.rearrange("p (t n) -> p t n", n=S),
                                in_=qT8.rearrange("(t p) n -> p t n", p=128))
            wk_b2T_sb = cp.tile([128, JT], f32, tag="wkb")
            nc.sync.dma_start(out=wk_b2T_sb, in_=wk_b2T)
            wk_sb = load_w8(wk8, "k", [nc.sync, nc.scalar])

            kT8_sb = acp.tile([128, DT * SEQ], f8, tag="kT")
            nc.sync.dma_start(out=kT8_sb.rearrange("p (t n) -> p t n", n=SEQ),
                              in_=kT8.rearrange("(t p) n -> p t n", p=128))
            wq_b2T_sb = cp.tile([128, JT], f32, tag="wqb")
            nc.sync.dma_start(out=wq_b2T_sb, in_=wq_b2T)

            # ---- persistent state ----
            # qpT8 interleaved: col block (dt*HEADS + h)*S, stored as 2*qp
            qpT8_sb = sp.tile([128, JT * S], f8, tag="qpT")       # 8KB/part
            kpT8_sb = sp.tile([128, JT * SEQ], f8, tag="kpT")     # 16KB/part
            vp_sb = sp.tile([128, TT * HD], b16, tag="vp")        # 32KB/part
            vp8_sb = sp.tile([128, TT * HD], f8, tag="vp8")       # 16KB/part
            oT_sb = sp.tile([128, DT * 2048], b16, tag="oT")      # 16KB/part
            fin32 = sp.tile([128, 2 * D], f32, tag="fin32")       # 4KB/part
            colrow_sb = sp.tile([1, 512], b16, tag="colrow")

            qview = qT8_sb.rearrange("p (t n) -> p t n", n=S)
            kview_in = kT8_sb.rearrange("p (t n) -> p t n", n=SEQ)

            # ---- phase A1: qpT8[j, s] = 2*(wk.T @ q + wk_b), fp8 DR ----
            for jt in range(JT):
                h, dt_of_j = divmod(jt, DT)
                ps = psA.tile([128, 512], f32, tag="psA")
                for p in range(NP):
                    nc.tensor.matmul(
                        ps[:, :S],
                        lhsT=w8slice(wk_sb, p, jt * 128),
                        rhs=qview[:, 2 * p:2 * p + 2, :],
                        start=(p == 0), stop=(p == NP - 1), perf_mode=DR)
                blk = dt_of_j * HEADS + h
                # PSUM->fp8 cast with bias: split DVE / scalar (gpsimd
                # cannot access PSUM)
                if jt % 2 == 0:
                    nc.vector.tensor_scalar(
                        qpT8_sb[:, blk * S:(blk + 1) * S], ps[:, :S],
                        1.0 / 32.0, wk_b2T_sb[:, jt:jt + 1],
                        op0=ALU.mult, op1=ALU.add)
                else:
                    nc.scalar.activation(
                        qpT8_sb[:, blk * S:(blk + 1) * S], ps[:, :S],
                        Act.Identity, bias=wk_b2T_sb[:, jt:jt + 1],
                        scale=1.0 / 32.0)

            # ---- phase A2: kpT8[j, t] = 2*(wq.T @ k + wq_b), fp8 DR ----
            wq_sb = load_w8(wq8, "q", [nc.sync, nc.scalar])
            vT_sb = acp.tile([128, DT * SEQ], b16, tag="vT")
            nc.scalar.dma_start(out=vT_sb.rearrange("p (t n) -> p t n", n=SEQ),
                                in_=vT.rearrange("(t p) n -> p t n", p=128))
            for jt in range(JT):
                ps = psA.tile([128, 512], f32, tag="psA")
                for p in range(NP):
                    nc.tensor.matmul(
                        ps,
                        lhsT=w8slice(wq_sb, p, jt * 128),
                        rhs=kview_in[:, 2 * p:2 * p + 2, :],
                        start=(p == 0), stop=(p == NP - 1), perf_mode=DR)
                if jt % 2 == 0:
                    nc.vector.tensor_scalar(
                        kpT8_sb[:, jt * SEQ:(jt + 1) * SEQ], ps,
                        1.0 / 32.0, wq_b2T_sb[:, jt:jt + 1],
                        op0=ALU.mult, op1=ALU.add)
                else:
                    nc.scalar.activation(
                        kpT8_sb[:, jt * SEQ:(jt + 1) * SEQ], ps,
                        Act.Identity, bias=wq_b2T_sb[:, jt:jt + 1],
                        scale=1.0 / 32.0)

            # ---- phase A3: vp[t, j] = v.T @ wv + wv_b, bf16 + fp8 copy ----
            wv_sb = load_w16(wvT, "v", [nc.sync, nc.scalar])
            wv_br_sb = cp.tile([128, HD], f32, tag="wvb")
            nc.scalar.dma_start(out=wv_br_sb, in_=wv_br)
            ones8_sb = cp.tile([128, 256], f8, tag="ones8")
            nc.sync.dma_start(out=ones8_sb, in_=ones8)
            consts_sb = cp.tile([128, 640], b16, tag="consts")
            nc.sync.dma_start(out=consts_sb, in_=consts)
            out_br_sb = cp.tile([128, D], f32, tag="outb")
            nc.sync.dma_start(out=out_br_sb, in_=out_br)
            for tt in range(TT):
                for js in range(HEADS):
                    ps = psA.tile([128, 512], f32, tag="psA")
                    for dt_ in range(DT):
                        nc.tensor.matmul(
                            ps,
                            lhsT=vT_sb[:, dt_ * SEQ + tt * 128:
                                       dt_ * SEQ + (tt + 1) * 128],
                            rhs=w16slice(wv_sb, dt_, js * 512, 512),
                            start=(dt_ == 0), stop=(dt_ == DT - 1))
                    c0 = tt * HD + js * 512
                    nc.vector.tensor_add(vp_sb[:, c0:c0 + 512], ps,
                                         wv_br_sb[:, js * 512:(js + 1) * 512])
                    nc.gpsimd.tensor_copy(vp8_sb[:, c0:c0 + 512],
                                          vp_sb[:, c0:c0 + 512])

            # ---- colsum(vp) over all 4096 keys, exact bf16 path ----
            # [1, 512] PE reductions with a ones column as the stationary op.
            pscol = psA.tile([1, 512], f32, tag="psA", name="pscol")
            ci = 0
            for g in range(HEADS):
                for tt in range(TT):
                    nc.tensor.matmul(
                        pscol,
                        lhsT=consts_sb[:, 0:1],
                        rhs=vp_sb[:, tt * HD + g * 512:tt * HD + (g + 1) * 512],
                        start=(ci == 0), stop=(ci == HEADS * TT - 1))
                    ci += 1
            nc.vector.tensor_copy(colrow_sb[0:1, :], pscol)

            # ---- phase B + pipelined out-projection, 4 l-slices ----
            kview = kpT8_sb.rearrange("p (j t) -> p j t", t=SEQ)
            qpview = qpT8_sb.rearrange("p (d hs) -> p d hs", hs=HEADS * S)
            vview = vp8_sb.rearrange("p (t j) -> p t j", j=HD)
            oview = ones8_sb.rearrange("p (two j) -> p two j", two=2)
            onesrow = consts_sb[0:1, 0:512]
            zconst = consts_sb[0:1, 512:640]

            prev_outproj = [None]

            for ls in range(LSLICES):
                h0 = 2 * ls
                psZ = psO.tile([128, 512], f32, tag="psO", name=f"psZ{ls}")
                po = [psO.tile([128, 512], f32, tag="psO", name=f"po{ls}_{i}")
                      for i in range(DT)]
                npair = HEADS * TT // 2  # 16
                # po/psZ accumulation MMs are emitted SKEWP pairs behind the
                # scores+exp of their chunks: at a slice start they block on
                # the previous slice's PSUM drain, and the PE queue is strict
                # FIFO - the skew puts independent scores work ahead of them.
                SKEWP = 2
                pending = []

                def emit_pair(pi, g, tt0, rp, psZ=psZ, po=po, npair=npair):
                    rv = rp.rearrange("p (two l) -> p two l", two=2)
                    if pi == 0:
                        # chain heads: +4096 into Z, +colsum into each po[et]
                        nc.tensor.matmul(psZ, lhsT=zconst, rhs=onesrow,
                                         start=True, stop=False)
                        for et in range(DT):
                            nc.tensor.matmul(
                                po[et],
                                lhsT=colrow_sb[0:1, et * 128:(et + 1) * 128],
                                rhs=onesrow, start=True, stop=False)
                    nc.tensor.matmul(psZ, lhsT=oview, rhs=rv,
                                     start=False, stop=(pi == npair - 1),
                                     perf_mode=DR)
                    for et in range(DT):
                        nc.tensor.matmul(
                            po[et],
                            lhsT=vview[:, tt0:tt0 + 2,
                                       g * 512 + et * 128:
                                       g * 512 + (et + 1) * 128],
                            rhs=rv, start=False, stop=(pi == npair - 1),
                            perf_mode=DR)

                rp_cur = [None]
                for g in range(HEADS):
                    for tt in range(TT):
                        ci = g * TT + tt
                        ps = psA.tile([128, 512], f32, tag="psA")
                        # scoresT[(g,tt), (h0..h0+1, s)] = 4*s_raw, fp8 DR
                        for p in range(NP):
                            nc.tensor.matmul(
                                ps,
                                lhsT=kview[:, g * DT + 2 * p:g * DT + 2 * p + 2,
                                           tt * 128:(tt + 1) * 128],
                                rhs=qpview[:, 2 * p:2 * p + 2,
                                           h0 * S:(h0 + 2) * S],
                                start=(p == 0), stop=(p == NP - 1),
                                perf_mode=DR)
                        ext = xp.tile([128, 512], b16, tag="ex")
                        nc.scalar.activation(ext, ps, Act.Exp, bias=0.0,
                                             scale=inv_sqrt_d / 4.0)
                        if tt % 2 == 0:
                            rp_cur[0] = ep.tile([128, 1024], f8, tag="rp",
                                                name=f"rp{ls}_{ci}")
                        nc.gpsimd.tensor_scalar_sub(
                            rp_cur[0][:, (tt % 2) * 512:(tt % 2) * 512 + 512],
                            ext, 1.0)
                        if tt % 2 == 1:
                            pending.append((ci // 2, g, tt - 1, rp_cur[0]))
                        if ci == 2 and prev_outproj[0] is not None:
                            prev_outproj[0]()
                            prev_outproj[0] = None
                        if len(pending) > SKEWP:
                            emit_pair(*pending.pop(0))
                for args in pending:
                    emit_pair(*args)

                # Z finalization + normalization: Z is fully reduced in psZ
                # (replicated across partitions); normalize po into oT.
                zr = zp.tile([128, 512], f32, tag="zr", name=f"zr{ls}")
                nc.vector.reciprocal(zr, psZ)
                for half in range(2):
                    for et in range(DT):
                        c0 = et * 2048 + ls * 512 + half * 256
                        nc.vector.tensor_mul(
                            oT_sb[:, c0:c0 + 256],
                            po[et][:, half * 256:(half + 1) * 256],
                            zr[:, half * 256:(half + 1) * 256])

                def make_outproj(ls=ls, h0=h0):
                    def outproj():
                        # out-projection contribution of this l-slice:
                        # c-tiles ct = h*DT+et for h in (h0, h0+1), bf16
                        ow_tiles = {}
                        for st in range(2):
                            psc = psA.tile([128, 512], f32, tag="psA",
                                           name=f"psc{ls}_{st}")
                            for ci2, ct in enumerate(
                                    range(h0 * DT, (h0 + 2) * DT)):
                                h, et = divmod(ct, DT)
                                if st == 0:
                                    ow_tiles[ct] = owp.tile(
                                        [128, D], b16, tag="ow",
                                        name=f"ow{ct}")
                                    nc.sync.dma_start(
                                        out=ow_tiles[ct],
                                        in_=owT[ct * 128:(ct + 1) * 128, :])
                                nc.tensor.matmul(
                                    psc,
                                    lhsT=oT_sb[:, et * 2048 + h * S + st * 128:
                                               et * 2048 + h * S +
                                               (st + 1) * 128],
                                    rhs=ow_tiles[ct],
                                    start=(ci2 == 0),
                                    stop=(ci2 == 2 * DT - 1))
                            if ls == 0:
                                nc.vector.tensor_add(
                                    fin32[:, st * D:(st + 1) * D],
                                    psc, out_br_sb)
                            else:
                                nc.vector.tensor_add(
                                    fin32[:, st * D:(st + 1) * D],
                                    psc, fin32[:, st * D:(st + 1) * D])
                    return outproj

                prev_outproj[0] = make_outproj()

            prev_outproj[0]()
            for st in range(2):
                nc.sync.dma_start(out=out[st * 128:(st + 1) * 128, :],
                                  in_=fin32[:, st * D:(st + 1) * D])

    nc.compile()
    return nc


def _get_program():
    if "nc" not in _CACHE:
        _CACHE["nc"] = _build_program()
    return _CACHE["nc"]


def _prep_shared(inputs):
    bf = NP_BF16
    e4 = NP_E4
    f32c = np.ascontiguousarray
    consts = np.ones((128, 640), np.float32)
    consts[:, 512:640] = 4096.0
    shared = {
        "wk8": f32c(np.asarray(inputs["wk_w"], np.float32).T * 64).astype(e4),
        "wq8": f32c(np.asarray(inputs["wq_w"], np.float32).T * 64).astype(e4),
        "wvT": f32c(np.asarray(inputs["wv_w"], np.float32).T).astype(bf),
        "owT": f32c(np.asarray(inputs["out_w"], np.float32).T).astype(bf),
        "wk_b2T": f32c(
            (2 * np.asarray(inputs["wk_b"], np.float32)).reshape(JT, 128).T),
        "wq_b2T": f32c(
            (2 * np.asarray(inputs["wq_b"], np.float32)).reshape(JT, 128).T),
        "wv_br": f32c(np.broadcast_to(
            np.asarray(inputs["wv_b"], np.float32)[None, :], (128, HD))),
        "out_br": f32c(np.broadcast_to(
            np.asarray(inputs["out_b"], np.float32)[None, :], (128, D))),
        "ones8": np.ones((128, 256), e4),
        "consts": consts.astype(bf),
    }
    return shared


def _make_in_maps(inputs):
    bf = NP_BF16
    e4 = NP_E4
    shared = _prep_shared(inputs)
    q = np.asarray(inputs["q"], np.float32)
    k = np.asarray(inputs["k"], np.float32)
    v = np.asarray(inputs["v"], np.float32)
    in_maps = []
    for core in range(NCORES):
        b, half = divmod(core, 2)
        m = dict(shared)
        m["qT8"] = np.ascontiguousarray(
            q[b, half * S:(half + 1) * S, :].T).astype(e4)
        m["kT8"] = np.ascontiguousarray(k[b].T).astype(e4)
        m["vT"] = np.ascontiguousarray(v[b].T).astype(bf)
        in_maps.append(m)
    return in_maps


def kernel(**inputs):
    from concourse.bass_utils import run_bass_kernel_spmd

    nc = _get_program()
    in_maps = _make_in_maps(inputs)
    res = run_bass_kernel_spmd(nc, in_maps, core_ids=list(range(NCORES)))
    _CACHE["last_results"] = res
    out = np.empty((BS, SEQ, D), np.float32)
    for core in range(NCORES):
        b, half = divmod(core, 2)
        out[b, half * S:(half + 1) * S, :] = res.results[core]["out"]
    return out


if __name__ == "__main__":
    rng = np.random.default_rng(0)
    fake = {
        "q": rng.standard_normal((BS, SEQ, D)).astype(np.float32),
        "k": rng.standard_normal((BS, SEQ, D)).astype(np.float32),
        "v": rng.standard_normal((BS, SEQ, D)).astype(np.float32),
        "wq_w": (rng.standard_normal((D * HEADS, D)) * 0.02).astype(np.float32),
        "wq_b": (rng.standard_normal((D * HEADS,)) * 0.02).astype(np.float32),
        "wk_w": (rng.standard_normal((D * HEADS, D)) * 0.02).astype(np.float32),
        "wk_b": (rng.standard_normal((D * HEADS,)) * 0.02).astype(np.float32),
        "wv_w": (rng.standard_normal((D * HEADS, D)) * 0.02).astype(np.float32),
        "wv_b": (rng.standard_normal((D * HEADS,)) * 0.02).astype(np.float32),
        "out_w": (rng.standard_normal((D, D * HEADS)) * 0.02).astype(np.float32),
        "out_b": (rng.standard_normal((D,)) * 0.02).astype(np.float32),
    }
    o = kernel(**fake)
    print("kernel ran, out shape", o.shape, "std", o.std())


# revision 11
# speedup vs baseline: 1.7104x; 1.1102x over previous
"""Trainium2 Bass kernel for nn_MultiHeadAttention_48086453846410.

Reference computation (heads folded into the sequence axis, softmax over the
FULL L = seq*heads key axis):
    qp = (q @ wk_w.T + wk_b).reshape(bs, L, d)   # note swapped wk/wq, faithful
    kp = (k @ wq_w.T + wq_b).reshape(bs, L, d)
    vp = (v @ wv_w.T + wv_b).reshape(bs, L, d)
    scores = qp @ kp.T / sqrt(d); attn = softmax(scores, -1)
    o = (attn @ vp).reshape(bs, seq, d*heads)
    out = o @ out_w.T + out_b

Sharding: 8 cores = (batch b in 0..3) x (seq half). Each core owns 256 query
seq positions of one batch (2048 query rows l' = h*256+s). Softmax is over
keys, so query rows are independent -> no collectives.

Speed strategy vs the bf16 baseline (which ran at ~95% of the bf16 PE
roofline): move the q/k/scores path to fp8-e4m3 DoubleRow matmuls (cost
model: 0.5 cycles/out-row with a K=256 contraction per instruction = 4x bf16
MAC throughput), while protecting accuracy on the value path:

 - q/k projections: fp8 DR (q,k,wk,wq quantized e4m3; weights pre-scaled x64
   on host; bias+rescale+fp8-cast done in ONE gpsimd tensor_scalar per tile,
   taking the scalar engine out of the A phases entirely).
 - scores: fp8 DR over qp8/kp8 (stored x2).  Attribution runs showed the
   whole q/k path tolerates e4m3 (~5e-3 each); the v path does NOT.
 - attn@v uses the decomposition  o*Z = colsum(vp) + sum_m r_m vp8_m  with
   r = exp(s)-1 (|r| ~ 0.2): the dominant mean signal comes from an EXACT
   bf16-path colsum (one cheap [1,512] PE reduction), and fp8 noise only
   rides on the small fluctuation term -> fp8 DR for the big matmul at
   bf16-level accuracy.  Z = 4096 + sum_m r_m comes free from a DR
   ones-matmul accumulating in its own PSUM bank (replaces the baseline's
   gpsimd Z chain + final ones-matmul).  The "+4096" and "+colsum" terms are
   injected as rank-1 bf16 matmuls that START each PSUM accumulation chain.
 - v projection, colsum, and out-projection stay bf16 (v-path e4m3 was
   measured at 1.4-2.4e-2 error alone -> too close to the 2e-2 gate).
 - exp runs on the scalar engine (bf16 out); r8 = ex-1 -> fp8 on gpsimd.

Measured numpy emulation of this exact dataflow: maxrel 9.0e-3 (gate 2e-2).
PE cycle budget ~273K cycles (~114us at 2.4GHz) vs ~720K for the baseline;
phase B is jointly PE/scalar/gpsimd bound.
"""

import math
import sys

for _p in ("/opt/trn_rl_repo",):
    if _p not in sys.path:
        sys.path.insert(0, _p)

import numpy as np
import ml_dtypes

BS, SEQ, D, HEADS = 4, 512, 512, 8
NCORES = 8
S = SEQ // 2            # 256 query seq rows per core
HD = HEADS * D          # 4096 projection dim
JT = HD // 128          # 32 tiles of the projection dim
DT = D // 128           # 4 tiles of the 512 contraction dim
NP = DT // 2            # 2 DoubleRow k-tile pairs over d
TT = SEQ // 128         # 4 key-seq tiles per head
LSLICES = 4             # l' = 2048 per core, processed in 4 slices of 512
NP_BF16 = ml_dtypes.bfloat16
NP_E4 = ml_dtypes.float8_e4m3

_CACHE = {}


def _build_program():
    from concourse import bacc
    import concourse.mybir as mybir
    import concourse.tile as tile
    from concourse.dt import dt

    f32 = dt.float32
    b16 = dt.bfloat16
    f8 = dt.float8e4
    Act = mybir.ActivationFunctionType
    DR = mybir.MatmulPerfMode.DoubleRow
    ALU = mybir.AluOpType

    nc = bacc.Bacc(None, target_bir_lowering=False, debug=False,
                   num_devices=NCORES)

    def din(name, shape, dty=b16):
        return nc.dram_tensor(name, shape, dty, kind="ExternalInput").ap()

    qT8 = din("qT8", [D, S], f8)           # q[b, half].T             (d, s)
    kT8 = din("kT8", [D, SEQ], f8)         # k[b].T                   (d, t)
    vT = din("vT", [D, SEQ])               # v[b].T  bf16             (d, t)
    wk8 = din("wk8", [D, HD], f8)          # 64*wk_w.T  e4m3          (d, j)
    wq8 = din("wq8", [D, HD], f8)          # 64*wq_w.T  e4m3          (d, j)
    wvT = din("wvT", [D, HD])              # wv_w.T  bf16             (d, j)
    owT = din("owT", [HD, D])              # out_w.T  bf16            (c, r)
    wk_b2T = din("wk_b2T", [128, JT], f32)  # (2*wk_b).reshape(JT,128).T
    wq_b2T = din("wq_b2T", [128, JT], f32)
    wv_br = din("wv_br", [128, HD], f32)    # wv_b replicated
    out_br = din("out_br", [128, D], f32)   # out_b replicated
    ones8 = din("ones8", [128, 256], f8)    # 1.0s: Z DoubleRow lhsT
    consts = din("consts", [128, 640])      # [:512]=1.0  [512:640]=4096.0
    out = nc.dram_tensor("out", [S, D], f32, kind="ExternalOutput").ap()

    inv_sqrt_d = 1.0 / math.sqrt(D)

    with tile.TileContext(nc) as tc:
        with (
            tc.tile_pool(name="const", bufs=1) as cp,
            tc.tile_pool(name="wqk", bufs=8) as wp,
            tc.tile_pool(name="wvp", bufs=16) as wvp,
            tc.tile_pool(name="acts", bufs=1) as acp,
            tc.tile_pool(name="state", bufs=1) as sp,
            tc.tile_pool(name="rpairs", bufs=6) as ep,
            tc.tile_pool(name="exs", bufs=4) as xp,
            tc.tile_pool(name="zrp", bufs=2) as zp,
            tc.tile_pool(name="owp", bufs=8) as owp,
            tc.tile_pool(name="psA", bufs=3, space="PSUM") as psA,
            tc.tile_pool(name="psO", bufs=5, space="PSUM") as psO,
        ):
            # ---- fp8 weight streaming: DoubleRow pair tiles ----
            # tile (p, wq) holds d rows [p*256, (p+1)*256) as [128, 2, 1024]:
            # partition = d within 128-block, dim1 = the two d-blocks of the
            # DR pair, dim2 = j window.  Emission order = consumption order.
            def load_w8(dram_t, nm, engines):
                tiles = {}
                i = 0
                for wq_i in range(4):
                    for p in range(NP):
                        t = wp.tile([128, 2048], f8, tag="w",
                                    name=f"w8_{nm}_{p}_{wq_i}")
                        engines[i % len(engines)].dma_start(
                            out=t.rearrange("p (two j) -> p two j", two=2),
                            in_=dram_t[p * 256:(p + 1) * 256,
                                       wq_i * 1024:(wq_i + 1) * 1024]
                            .rearrange("(two p) j -> p two j", p=128))
                        tiles[(p, wq_i)] = t
                        i += 1
                return tiles

            def w8slice(tiles, p, j0):
                wq_i, off = divmod(j0, 1024)
                return tiles[(p, wq_i)].rearrange(
                    "p (two j) -> p two j", two=2)[:, :, off:off + 128]

            # bf16 quarter-tile streaming for wv (baseline scheme)
            def load_w16(dram_t, nm, engines):
                tiles = {}
                i = 0
                for wq_i in range(4):
                    for dt_ in range(DT):
                        t = wp.tile([128, 1024], b16, tag="w",
                                    name=f"w_{nm}_{dt_}_{wq_i}")
                        engines[i % len(engines)].dma_start(
                            out=t,
                            in_=dram_t[dt_ * 128:(dt_ + 1) * 128,
                                       wq_i * 1024:(wq_i + 1) * 1024])
                        tiles[(dt_, wq_i)] = t
                        i += 1
                return tiles

            def w16slice(tiles, dt_, j0, width):
                wq_i, off = divmod(j0, 1024)
                return tiles[(dt_, wq_i)][:, off:off + width]

            # phase-A1 critical path first: qT8 (small) then wk weights
            qT8_sb = acp.tile([128, DT * S], f8, tag="qT")
            nc.gpsimd.dma_start(out=qT8_sb.rearrange("p (t n) -> p t n", n=S),
                                in_=qT8.rearrange("(t p) n -> p t n", p=128))
            wk_b2T_sb = cp.tile([128, JT], f32, tag="wkb")
            nc.sync.dma_start(out=wk_b2T_sb, in_=wk_b2T)
            wk_sb = load_w8(wk8, "k", [nc.sync, nc.scalar])

            kT8_sb = acp.tile([128, DT * SEQ], f8, tag="kT")
            nc.sync.dma_start(out=kT8_sb.rearrange("p (t n) -> p t n", n=SEQ),
                              in_=kT8.rearrange("(t p) n -> p t n", p=128))
            wq_b2T_sb = cp.tile([128, JT], f32, tag="wqb")
            nc.sync.dma_start(out=wq_b2T_sb, in_=wq_b2T)

            # ---- persistent state ----
            # qpT8 interleaved: col block (dt*HEADS + h)*S, stored as 2*qp
            qpT8_sb = sp.tile([128, JT * S], f8, tag="qpT")       # 8KB/part
            kpT8_sb = sp.tile([128, JT * SEQ], f8, tag="kpT")     # 16KB/part
            vp_sb = sp.tile([128, TT * HD], b16, tag="vp")        # 32KB/part
            vp8_sb = sp.tile([128, TT * HD], f8, tag="vp8")       # 16KB/part
            oT_sb = sp.tile([128, DT * 2048], b16, tag="oT")      # 16KB/part
            fin32 = sp.tile([128, 2 * D], f32, tag="fin32")       # 4KB/part
            colrow_sb = sp.tile([1, 512], b16, tag="colrow")

            qview = qT8_sb.rearrange("p (t n) -> p t n", n=S)
            kview_in = kT8_sb.rearrange("p (t n) -> p t n", n=SEQ)

            # ---- phase A1: qpT8[j, s] = 2*(wk.T @ q + wk_b), fp8 DR ----
            for jt in range(JT):
                h, dt_of_j = divmod(jt, DT)
                ps = psA.tile([128, 512], f32, tag="psA")
                for p in range(NP):
                    nc.tensor.matmul(
                        ps[:, :S],
                        lhsT=w8slice(wk_sb, p, jt * 128),
                        rhs=qview[:, 2 * p:2 * p + 2, :],
                        start=(p == 0), stop=(p == NP - 1), perf_mode=DR)
                blk = dt_of_j * HEADS + h
                # PSUM->fp8 cast with bias: only DVE/Act can read PSUM;
                # 5:3 DVE:Act split (Act carries all of A2's casts later)
                if jt % 8 < 5:
                    nc.vector.tensor_scalar(
                        qpT8_sb[:, blk * S:(blk + 1) * S], ps[:, :S],
                        1.0 / 32.0, wk_b2T_sb[:, jt:jt + 1],
                        op0=ALU.mult, op1=ALU.add)
                else:
                    nc.scalar.activation(
                        qpT8_sb[:, blk * S:(blk + 1) * S], ps[:, :S],
                        Act.Identity, bias=wk_b2T_sb[:, jt:jt + 1],
                        scale=1.0 / 32.0)

            # ---- phases A2+A3 merged: one A2 chunk + one A3 tile per step
            # so PE (both), Act (A2 casts), DVE (A3 adds) and Pool (vp8
            # copies) all run concurrently instead of phase-serial.
            # A2: kpT8[j, t] = 2*(wq.T @ k + wq_b), fp8 DR   (cast on Act)
            # A3: vp[t, j] = v.T @ wv + wv_b, bf16           (add on DVE)
            vT_sb = acp.tile([128, DT * SEQ], b16, tag="vT")
            nc.gpsimd.dma_start(out=vT_sb.rearrange("p (t n) -> p t n", n=SEQ),
                                in_=vT.rearrange("(t p) n -> p t n", p=128))
            wv_br_sb = cp.tile([128, HD], f32, tag="wvb")
            nc.sync.dma_start(out=wv_br_sb, in_=wv_br)
            ones8_sb = cp.tile([128, 256], f8, tag="ones8")
            nc.sync.dma_start(out=ones8_sb, in_=ones8)
            consts_sb = cp.tile([128, 640], b16, tag="consts")
            nc.sync.dma_start(out=consts_sb, in_=consts)
            out_br_sb = cp.tile([128, D], f32, tag="outb")
            nc.sync.dma_start(out=out_br_sb, in_=out_br)

            # stream wq8 pair-tiles and wv quarter-tiles in first-use order
            wq_sb, wv_sb = {}, {}
            eng = [nc.sync, nc.scalar]
            emits = []  # (kind, key) in first-use order of the merged loop
            emits += [("q", (p, 0)) for p in range(NP)]
            emits += [("v", (dt_, 0)) for dt_ in range(DT)]
            for wq_i in range(1, 4):
                emits += [("v", (dt_, wq_i)) for dt_ in range(DT)]
            for wq_i in range(1, 4):
                emits += [("q", (p, wq_i)) for p in range(NP)]
            for i, (kind, key) in enumerate(emits):
                e = eng[i % 2]
                if kind == "q":
                    p, wq_i = key
                    t = wp.tile([128, 2048], f8, tag="w",
                                name=f"w8_q_{p}_{wq_i}")
                    e.dma_start(
                        out=t.rearrange("p (two j) -> p two j", two=2),
                        in_=wq8[p * 256:(p + 1) * 256,
                                wq_i * 1024:(wq_i + 1) * 1024]
                        .rearrange("(two p) j -> p two j", p=128))
                    wq_sb[key] = t
                else:
                    dt_, wq_i = key
                    t = wvp.tile([128, 1024], b16, tag="wv",
                                 name=f"w_v_{dt_}_{wq_i}")
                    e.dma_start(out=t,
                                in_=wvT[dt_ * 128:(dt_ + 1) * 128,
                                        wq_i * 1024:(wq_i + 1) * 1024])
                    wv_sb[key] = t

            for i in range(JT):
                # A2 chunk jt=i
                ps2 = psA.tile([128, 512], f32, tag="psA", name=f"psA2_{i}")
                for p in range(NP):
                    nc.tensor.matmul(
                        ps2,
                        lhsT=w8slice(wq_sb, p, i * 128),
                        rhs=kview_in[:, 2 * p:2 * p + 2, :],
                        start=(p == 0), stop=(p == NP - 1), perf_mode=DR)
                nc.scalar.activation(
                    kpT8_sb[:, i * SEQ:(i + 1) * SEQ], ps2,
                    Act.Identity, bias=wq_b2T_sb[:, i:i + 1],
                    scale=1.0 / 32.0)
                # A3 tile (tt, js)
                tt, js = divmod(i, HEADS)
                ps3 = psA.tile([128, 512], f32, tag="psA", name=f"psA3_{i}")
                for dt_ in range(DT):
                    nc.tensor.matmul(
                        ps3,
                        lhsT=vT_sb[:, dt_ * SEQ + tt * 128:
                                   dt_ * SEQ + (tt + 1) * 128],
                        rhs=w16slice(wv_sb, dt_, js * 512, 512),
                        start=(dt_ == 0), stop=(dt_ == DT - 1))
                c0 = tt * HD + js * 512
                nc.vector.tensor_add(vp_sb[:, c0:c0 + 512], ps3,
                                     wv_br_sb[:, js * 512:(js + 1) * 512])
                nc.gpsimd.tensor_copy(vp8_sb[:, c0:c0 + 512],
                                      vp_sb[:, c0:c0 + 512])

            # ---- colsum(vp) over all 4096 keys, exact bf16 path ----
            # [1, 512] PE reductions with a ones column as the stationary op.
            pscol = psA.tile([1, 512], f32, tag="psA", name="pscol")
            ci = 0
            for g in range(HEADS):
                for tt in range(TT):
                    nc.tensor.matmul(
                        pscol,
                        lhsT=consts_sb[:, 0:1],
                        rhs=vp_sb[:, tt * HD + g * 512:tt * HD + (g + 1) * 512],
                        start=(ci == 0), stop=(ci == HEADS * TT - 1))
                    ci += 1
            nc.vector.tensor_copy(colrow_sb[0:1, :], pscol)

            # ---- phase B + pipelined out-projection, 4 l-slices ----
            kview = kpT8_sb.rearrange("p (j t) -> p j t", t=SEQ)
            qpview = qpT8_sb.rearrange("p (d hs) -> p d hs", hs=HEADS * S)
            vview = vp8_sb.rearrange("p (t j) -> p t j", j=HD)
            oview = ones8_sb.rearrange("p (two j) -> p two j", two=2)
            onesrow = consts_sb[0:1, 0:512]
            zconst = consts_sb[0:1, 512:640]

            prev_outproj = [None]

            for ls in range(LSLICES):
                h0 = 2 * ls
                psZ = psO.tile([128, 512], f32, tag="psO", name=f"psZ{ls}")
                po = [psO.tile([128, 512], f32, tag="psO", name=f"po{ls}_{i}")
                      for i in range(DT)]
                npair = HEADS * TT // 2  # 16
                # po/psZ accumulation MMs are emitted SKEWP pairs behind the
                # scores+exp of their chunks: at a slice start they block on
                # the previous slice's PSUM drain, and the PE queue is strict
                # FIFO - the skew puts independent scores work ahead of them.
                SKEWP = 2
                pending = []

                def emit_pair(pi, g, tt0, rp, psZ=psZ, po=po, npair=npair):
                    rv = rp.rearrange("p (two l) -> p two l", two=2)
                    if pi == 0:
                        # chain heads: +4096 into Z, +colsum into each po[et]
                        nc.tensor.matmul(psZ, lhsT=zconst, rhs=onesrow,
                                         start=True, stop=False)
                        for et in range(DT):
                            nc.tensor.matmul(
                                po[et],
                                lhsT=colrow_sb[0:1, et * 128:(et + 1) * 128],
                                rhs=onesrow, start=True, stop=False)
                    nc.tensor.matmul(psZ, lhsT=oview, rhs=rv,
                                     start=False, stop=(pi == npair - 1),
                                     perf_mode=DR)
                    for et in range(DT):
                        nc.tensor.matmul(
                            po[et],
                            lhsT=vview[:, tt0:tt0 + 2,
                                       g * 512 + et * 128:
                                       g * 512 + (et + 1) * 128],
                            rhs=rv, start=False, stop=(pi == npair - 1),
                            perf_mode=DR)

                rp_cur = [None]
                for g in range(HEADS):
                    for tt in range(TT):
                        ci = g * TT + tt
                        ps = psA.tile([128, 512], f32, tag="psA")
                        # scoresT[(g,tt), (h0..h0+1, s)] = 4*s_raw, fp8 DR
                        for p in range(NP):
                            nc.tensor.matmul(
                                ps,
                                lhsT=kview[:, g * DT + 2 * p:g * DT + 2 * p + 2,
                                           tt * 128:(tt + 1) * 128],
                                rhs=qpview[:, 2 * p:2 * p + 2,
                                           h0 * S:(h0 + 2) * S],
                                start=(p == 0), stop=(p == NP - 1),
                                perf_mode=DR)
                        ext = xp.tile([128, 512], b16, tag="ex")
                        nc.scalar.activation(ext, ps, Act.Exp, bias=0.0,
                                             scale=inv_sqrt_d / 4.0)
                        if tt % 2 == 0:
                            rp_cur[0] = ep.tile([128, 1024], f8, tag="rp",
                                                name=f"rp{ls}_{ci}")
                        # r8 = ex - 1 -> fp8; 5:3 DVE:Pool split (Pool ops
                        # cost 806ns vs DVE 593ns in the cost model, and Act
                        # is pinned by the exps)
                        sub_eng = nc.vector if ci % 8 < 5 else nc.gpsimd
                        sub_eng.tensor_scalar_sub(
                            rp_cur[0][:, (tt % 2) * 512:(tt % 2) * 512 + 512],
                            ext, 1.0)
                        if tt % 2 == 1:
                            pending.append((ci // 2, g, tt - 1, rp_cur[0]))
                        if ci == 2 and prev_outproj[0] is not None:
                            prev_outproj[0]()
                            prev_outproj[0] = None
                        if len(pending) > SKEWP:
                            emit_pair(*pending.pop(0))
                for args in pending:
                    emit_pair(*args)

                # Z finalization + normalization: Z is fully reduced in psZ
                # (replicated across partitions); normalize po into oT.
                zr = zp.tile([128, 512], f32, tag="zr", name=f"zr{ls}")
                nc.vector.reciprocal(zr, psZ)
                for half in range(2):
                    for et in range(DT):
                        c0 = et * 2048 + ls * 512 + half * 256
                        nc.vector.tensor_mul(
                            oT_sb[:, c0:c0 + 256],
                            po[et][:, half * 256:(half + 1) * 256],
                            zr[:, half * 256:(half + 1) * 256])

                def make_outproj(ls=ls, h0=h0):
                    def outproj():
                        # out-projection contribution of this l-slice:
                        # c-tiles ct = h*DT+et for h in (h0, h0+1), bf16
                        ow_tiles = {}
                        for st in range(2):
                            psc = psA.tile([128, 512], f32, tag="psA",
                                           name=f"psc{ls}_{st}")
                            for ci2, ct in enumerate(
                                    range(h0 * DT, (h0 + 2) * DT)):
                                h, et = divmod(ct, DT)
                                if st == 0:
                                    ow_tiles[ct] = owp.tile(
                                        [128, D], b16, tag="ow",
                                        name=f"ow{ct}")
                                    nc.sync.dma_start(
                                        out=ow_tiles[ct],
                                        in_=owT[ct * 128:(ct + 1) * 128, :])
                                nc.tensor.matmul(
                                    psc,
                                    lhsT=oT_sb[:, et * 2048 + h * S + st * 128:
                                               et * 2048 + h * S +
                                               (st + 1) * 128],
                                    rhs=ow_tiles[ct],
                                    start=(ci2 == 0),
                                    stop=(ci2 == 2 * DT - 1))
                            if ls == 0:
                                nc.vector.tensor_add(
                                    fin32[:, st * D:(st + 1) * D],
                                    psc, out_br_sb)
                            else:
                                nc.vector.tensor_add(
                                    fin32[:, st * D:(st + 1) * D],
                                    psc, fin32[:, st * D:(st + 1) * D])
                    return outproj

                prev_outproj[0] = make_outproj()

            prev_outproj[0]()
            for st in range(2):
                nc.sync.dma_start(out=out[st * 128:(st + 1) * 128, :],
                                  in_=fin32[:, st * D:(st + 1) * D])

    nc.compile()
    return nc


def _get_program():
    if "nc" not in _CACHE:
        _CACHE["nc"] = _build_program()
    return _CACHE["nc"]


def _prep_shared(inputs):
    bf = NP_BF16
    e4 = NP_E4
    f32c = np.ascontiguousarray
    consts = np.ones((128, 640), np.float32)
    consts[:, 512:640] = 4096.0
    shared = {
        "wk8": f32c(np.asarray(inputs["wk_w"], np.float32).T * 64).astype(e4),
        "wq8": f32c(np.asarray(inputs["wq_w"], np.float32).T * 64).astype(e4),
        "wvT": f32c(np.asarray(inputs["wv_w"], np.float32).T).astype(bf),
        "owT": f32c(np.asarray(inputs["out_w"], np.float32).T).astype(bf),
        "wk_b2T": f32c(
            (2 * np.asarray(inputs["wk_b"], np.float32)).reshape(JT, 128).T),
        "wq_b2T": f32c(
            (2 * np.asarray(inputs["wq_b"], np.float32)).reshape(JT, 128).T),
        "wv_br": f32c(np.broadcast_to(
            np.asarray(inputs["wv_b"], np.float32)[None, :], (128, HD))),
        "out_br": f32c(np.broadcast_to(
            np.asarray(inputs["out_b"], np.float32)[None, :], (128, D))),
        "ones8": np.ones((128, 256), e4),
        "consts": consts.astype(bf),
    }
    return shared


def _make_in_maps(inputs):
    bf = NP_BF16
    e4 = NP_E4
    shared = _prep_shared(inputs)
    q = np.asarray(inputs["q"], np.float32)
    k = np.asarray(inputs["k"], np.float32)
    v = np.asarray(inputs["v"], np.float32)
    in_maps = []
    for core in range(NCORES):
        b, half = divmod(core, 2)
        m = dict(shared)
        m["qT8"] = np.ascontiguousarray(
            q[b, half * S:(half + 1) * S, :].T).astype(e4)
        m["kT8"] = np.ascontiguousarray(k[b].T).astype(e4)
        m["vT"] = np.ascontiguousarray(v[b].T).astype(bf)
        in_maps.append(m)
    return in_maps


def kernel(**inputs):
    from concourse.bass_utils import run_bass_kernel_spmd

    nc = _get_program()
    in_maps = _make_in_maps(inputs)
    res = run_bass_kernel_spmd(nc, in_maps, core_ids=list(range(NCORES)))
    _CACHE["last_results"] = res
    out = np.empty((BS, SEQ, D), np.float32)
    for core in range(NCORES):
        b, half = divmod(core, 2)
        out[b, half * S:(half + 1) * S, :] = res.results[core]["out"]
    return out


if __name__ == "__main__":
    rng = np.random.default_rng(0)
    fake = {
        "q": rng.standard_normal((BS, SEQ, D)).astype(np.float32),
        "k": rng.standard_normal((BS, SEQ, D)).astype(np.float32),
        "v": rng.standard_normal((BS, SEQ, D)).astype(np.float32),
        "wq_w": (rng.standard_normal((D * HEADS, D)) * 0.02).astype(np.float32),
        "wq_b": (rng.standard_normal((D * HEADS,)) * 0.02).astype(np.float32),
        "wk_w": (rng.standard_normal((D * HEADS, D)) * 0.02).astype(np.float32),
        "wk_b": (rng.standard_normal((D * HEADS,)) * 0.02).astype(np.float32),
        "wv_w": (rng.standard_normal((D * HEADS, D)) * 0.02).astype(np.float32),
        "wv_b": (rng.standard_normal((D * HEADS,)) * 0.02).astype(np.float32),
        "out_w": (rng.standard_normal((D, D * HEADS)) * 0.02).astype(np.float32),
        "out_b": (rng.standard_normal((D,)) * 0.02).astype(np.float32),
    }
    o = kernel(**fake)
    print("kernel ran, out shape", o.shape, "std", o.std())


# revision 17
# speedup vs baseline: 1.7655x; 1.0323x over previous
"""Trainium2 Bass kernel for nn_MultiHeadAttention_48086453846410.

Reference computation (heads folded into the sequence axis, softmax over the
FULL L = seq*heads key axis):
    qp = (q @ wk_w.T + wk_b).reshape(bs, L, d)   # note swapped wk/wq, faithful
    kp = (k @ wq_w.T + wq_b).reshape(bs, L, d)
    vp = (v @ wv_w.T + wv_b).reshape(bs, L, d)
    scores = qp @ kp.T / sqrt(d); attn = softmax(scores, -1)
    o = (attn @ vp).reshape(bs, seq, d*heads)
    out = o @ out_w.T + out_b

Sharding: 8 cores = (batch b in 0..3) x (seq half). Each core owns 256 query
seq positions of one batch (2048 query rows l' = h*256+s). Softmax is over
keys, so query rows are independent -> no collectives.

Speed strategy vs the bf16 baseline (which ran at ~95% of the bf16 PE
roofline): move the q/k/scores path to fp8-e4m3 DoubleRow matmuls (cost
model: 0.5 cycles/out-row with a K=256 contraction per instruction = 4x bf16
MAC throughput), while protecting accuracy on the value path:

 - q/k projections: fp8 DR (q,k,wk,wq quantized e4m3; weights pre-scaled x64
   on host; bias+rescale+fp8-cast done in ONE gpsimd tensor_scalar per tile,
   taking the scalar engine out of the A phases entirely).
 - scores: fp8 DR over qp8/kp8 (stored x2).  Attribution runs showed the
   whole q/k path tolerates e4m3 (~5e-3 each); the v path does NOT.
 - attn@v uses the decomposition  o*Z = colsum(vp) + sum_m r_m vp8_m  with
   r = exp(s)-1 (|r| ~ 0.2): the dominant mean signal comes from an EXACT
   bf16-path colsum (one cheap [1,512] PE reduction), and fp8 noise only
   rides on the small fluctuation term -> fp8 DR for the big matmul at
   bf16-level accuracy.  Z = 4096 + sum_m r_m comes free from a DR
   ones-matmul accumulating in its own PSUM bank (replaces the baseline's
   gpsimd Z chain + final ones-matmul).  The "+4096" and "+colsum" terms are
   injected as rank-1 bf16 matmuls that START each PSUM accumulation chain.
 - v projection, colsum, and out-projection stay bf16 (v-path e4m3 was
   measured at 1.4-2.4e-2 error alone -> too close to the 2e-2 gate).
 - exp runs on the scalar engine (bf16 out); r8 = ex-1 -> fp8 on gpsimd.

Measured numpy emulation of this exact dataflow: maxrel 9.0e-3 (gate 2e-2).
PE cycle budget ~273K cycles (~114us at 2.4GHz) vs ~720K for the baseline;
phase B is jointly PE/scalar/gpsimd bound.
"""

import math
import sys

for _p in ("/opt/trn_rl_repo",):
    if _p not in sys.path:
        sys.path.insert(0, _p)

import numpy as np
import ml_dtypes

BS, SEQ, D, HEADS = 4, 512, 512, 8
NCORES = 8
S = SEQ // 2            # 256 query seq rows per core
HD = HEADS * D          # 4096 projection dim
JT = HD // 128          # 32 tiles of the projection dim
DT = D // 128           # 4 tiles of the 512 contraction dim
NP = DT // 2            # 2 DoubleRow k-tile pairs over d
TT = SEQ // 128         # 4 key-seq tiles per head
LSLICES = 4             # l' = 2048 per core, processed in 4 slices of 512
NP_BF16 = ml_dtypes.bfloat16
NP_E4 = ml_dtypes.float8_e4m3

_CACHE = {}


def _build_program():
    from concourse import bacc
    import concourse.mybir as mybir
    import concourse.tile as tile
    from concourse.dt import dt

    f32 = dt.float32
    b16 = dt.bfloat16
    f8 = dt.float8e4
    Act = mybir.ActivationFunctionType
    DR = mybir.MatmulPerfMode.DoubleRow
    ALU = mybir.AluOpType

    nc = bacc.Bacc(None, target_bir_lowering=False, debug=False,
                   num_devices=NCORES)

    def din(name, shape, dty=b16):
        return nc.dram_tensor(name, shape, dty, kind="ExternalInput").ap()

    qT8 = din("qT8", [D, S], f8)           # q[b, half].T             (d, s)
    kT8 = din("kT8", [D, SEQ], f8)         # k[b].T                   (d, t)
    vT = din("vT", [D, SEQ])               # v[b].T  bf16             (d, t)
    wk8 = din("wk8", [D, HD], f8)          # 64*wk_w.T  e4m3          (d, j)
    wq8 = din("wq8", [D, HD], f8)          # 64*wq_w.T  e4m3          (d, j)
    wvT = din("wvT", [D, HD])              # wv_w.T  bf16             (d, j)
    owT = din("owT", [HD, D])              # out_w.T  bf16            (c, r)
    wk_b2T = din("wk_b2T", [128, JT], f32)  # (2*wk_b).reshape(JT,128).T
    wq_b2T = din("wq_b2T", [128, JT], f32)
    wv_br = din("wv_br", [128, HD], f32)    # wv_b replicated
    out_br = din("out_br", [128, D], f32)   # out_b replicated
    ones8 = din("ones8", [128, 256], f8)    # 1.0s: Z DoubleRow lhsT
    consts = din("consts", [128, 640])      # [:512]=1.0  [512:640]=4096.0
    out = nc.dram_tensor("out", [S, D], f32, kind="ExternalOutput").ap()

    inv_sqrt_d = 1.0 / math.sqrt(D)

    with tile.TileContext(nc) as tc:
        with (
            tc.tile_pool(name="const", bufs=1) as cp,
            tc.tile_pool(name="wqk", bufs=8) as wp,
            tc.tile_pool(name="wvp", bufs=16) as wvp,
            tc.tile_pool(name="acts", bufs=1) as acp,
            tc.tile_pool(name="state", bufs=1) as sp,
            tc.tile_pool(name="rpairs", bufs=8) as ep,
            tc.tile_pool(name="exs", bufs=6) as xp,
            tc.tile_pool(name="zrp", bufs=2) as zp,
            tc.tile_pool(name="owp", bufs=8) as owp,
            tc.tile_pool(name="psA", bufs=3, space="PSUM") as psA,
            tc.tile_pool(name="psO", bufs=5, space="PSUM") as psO,
        ):
            # ---- fp8 weight streaming: DoubleRow pair tiles ----
            # tile (p, wq) holds d rows [p*256, (p+1)*256) as [128, 2, 1024]:
            # partition = d within 128-block, dim1 = the two d-blocks of the
            # DR pair, dim2 = j window.  Emission order = consumption order.
            def load_w8(dram_t, nm, engines):
                tiles = {}
                i = 0
                for wq_i in range(4):
                    for p in range(NP):
                        t = wp.tile([128, 2048], f8, tag="w",
                                    name=f"w8_{nm}_{p}_{wq_i}")
                        engines[i % len(engines)].dma_start(
                            out=t.rearrange("p (two j) -> p two j", two=2),
                            in_=dram_t[p * 256:(p + 1) * 256,
                                       wq_i * 1024:(wq_i + 1) * 1024]
                            .rearrange("(two p) j -> p two j", p=128))
                        tiles[(p, wq_i)] = t
                        i += 1
                return tiles

            def w8slice(tiles, p, j0):
                wq_i, off = divmod(j0, 1024)
                return tiles[(p, wq_i)].rearrange(
                    "p (two j) -> p two j", two=2)[:, :, off:off + 128]

            # bf16 quarter-tile streaming for wv (baseline scheme)
            def load_w16(dram_t, nm, engines):
                tiles = {}
                i = 0
                for wq_i in range(4):
                    for dt_ in range(DT):
                        t = wp.tile([128, 1024], b16, tag="w",
                                    name=f"w_{nm}_{dt_}_{wq_i}")
                        engines[i % len(engines)].dma_start(
                            out=t,
                            in_=dram_t[dt_ * 128:(dt_ + 1) * 128,
                                       wq_i * 1024:(wq_i + 1) * 1024])
                        tiles[(dt_, wq_i)] = t
                        i += 1
                return tiles

            def w16slice(tiles, dt_, j0, width):
                wq_i, off = divmod(j0, 1024)
                return tiles[(dt_, wq_i)][:, off:off + width]

            # phase-A1 critical path first: qT8 (small) then wk weights
            qT8_sb = acp.tile([128, DT * S], f8, tag="qT")
            nc.gpsimd.dma_start(out=qT8_sb.rearrange("p (t n) -> p t n", n=S),
                                in_=qT8.rearrange("(t p) n -> p t n", p=128))
            wk_b2T_sb = cp.tile([128, JT], f32, tag="wkb")
            nc.sync.dma_start(out=wk_b2T_sb, in_=wk_b2T)
            wk_sb = load_w8(wk8, "k", [nc.sync, nc.scalar])

            kT8_sb = acp.tile([128, DT * SEQ], f8, tag="kT")
            nc.sync.dma_start(out=kT8_sb.rearrange("p (t n) -> p t n", n=SEQ),
                              in_=kT8.rearrange("(t p) n -> p t n", p=128))
            wq_b2T_sb = cp.tile([128, JT], f32, tag="wqb")
            nc.sync.dma_start(out=wq_b2T_sb, in_=wq_b2T)

            # ---- persistent state ----
            # qpT8 interleaved: col block (dt*HEADS + h)*S, stored as 2*qp
            qpT8_sb = sp.tile([128, JT * S], f8, tag="qpT")       # 8KB/part
            kpT8_sb = sp.tile([128, JT * SEQ], f8, tag="kpT")     # 16KB/part
            vp_sb = sp.tile([128, TT * HD], b16, tag="vp")        # 32KB/part
            vp8_sb = sp.tile([128, TT * HD], f8, tag="vp8")       # 16KB/part
            oT_sb = sp.tile([128, DT * 2048], b16, tag="oT")      # 16KB/part
            fin32 = sp.tile([128, 2 * D], f32, tag="fin32")       # 4KB/part
            colrow_sb = sp.tile([1, 512], b16, tag="colrow")

            qview = qT8_sb.rearrange("p (t n) -> p t n", n=S)
            kview_in = kT8_sb.rearrange("p (t n) -> p t n", n=SEQ)

            # ---- phase A1: qpT8[j, s] = 2*(wk.T @ q + wk_b), fp8 DR ----
            for jt in range(JT):
                h, dt_of_j = divmod(jt, DT)
                ps = psA.tile([128, 512], f32, tag="psA")
                for p in range(NP):
                    nc.tensor.matmul(
                        ps[:, :S],
                        lhsT=w8slice(wk_sb, p, jt * 128),
                        rhs=qview[:, 2 * p:2 * p + 2, :],
                        start=(p == 0), stop=(p == NP - 1), perf_mode=DR)
                blk = dt_of_j * HEADS + h
                # PSUM->fp8 cast with bias: only DVE/Act can read PSUM;
                # 5:3 DVE:Act split (Act carries all of A2's casts later)
                if jt % 8 < 5:
                    nc.vector.tensor_scalar(
                        qpT8_sb[:, blk * S:(blk + 1) * S], ps[:, :S],
                        1.0 / 32.0, wk_b2T_sb[:, jt:jt + 1],
                        op0=ALU.mult, op1=ALU.add)
                else:
                    nc.scalar.activation(
                        qpT8_sb[:, blk * S:(blk + 1) * S], ps[:, :S],
                        Act.Identity, bias=wk_b2T_sb[:, jt:jt + 1],
                        scale=1.0 / 32.0)

            # ---- phases A2+A3 merged: one A2 chunk + one A3 tile per step
            # so PE (both), Act (A2 casts), DVE (A3 adds) and Pool (vp8
            # copies) all run concurrently instead of phase-serial.
            # A2: kpT8[j, t] = 2*(wq.T @ k + wq_b), fp8 DR   (cast on Act)
            # A3: vp[t, j] = v.T @ wv + wv_b, bf16           (add on DVE)
            vT_sb = acp.tile([128, DT * SEQ], b16, tag="vT")
            nc.gpsimd.dma_start(out=vT_sb.rearrange("p (t n) -> p t n", n=SEQ),
                                in_=vT.rearrange("(t p) n -> p t n", p=128))
            wv_br_sb = cp.tile([128, HD], f32, tag="wvb")
            nc.sync.dma_start(out=wv_br_sb, in_=wv_br)
            ones8_sb = cp.tile([128, 256], f8, tag="ones8")
            nc.sync.dma_start(out=ones8_sb, in_=ones8)
            consts_sb = cp.tile([128, 640], b16, tag="consts")
            nc.sync.dma_start(out=consts_sb, in_=consts)
            out_br_sb = cp.tile([128, D], f32, tag="outb")
            nc.sync.dma_start(out=out_br_sb, in_=out_br)

            # stream wq8 pair-tiles and wv quarter-tiles in first-use order
            wq_sb, wv_sb = {}, {}
            eng = [nc.sync, nc.scalar]
            emits = []  # (kind, key) in first-use order of the merged loop
            emits += [("q", (p, 0)) for p in range(NP)]
            emits += [("v", (dt_, 0)) for dt_ in range(DT)]
            for wq_i in range(1, 4):
                emits += [("v", (dt_, wq_i)) for dt_ in range(DT)]
            for wq_i in range(1, 4):
                emits += [("q", (p, wq_i)) for p in range(NP)]
            for i, (kind, key) in enumerate(emits):
                e = eng[i % 2]
                if kind == "q":
                    p, wq_i = key
                    t = wp.tile([128, 2048], f8, tag="w",
                                name=f"w8_q_{p}_{wq_i}")
                    e.dma_start(
                        out=t.rearrange("p (two j) -> p two j", two=2),
                        in_=wq8[p * 256:(p + 1) * 256,
                                wq_i * 1024:(wq_i + 1) * 1024]
                        .rearrange("(two p) j -> p two j", p=128))
                    wq_sb[key] = t
                else:
                    dt_, wq_i = key
                    t = wvp.tile([128, 1024], b16, tag="wv",
                                 name=f"w_v_{dt_}_{wq_i}")
                    e.dma_start(out=t,
                                in_=wvT[dt_ * 128:(dt_ + 1) * 128,
                                        wq_i * 1024:(wq_i + 1) * 1024])
                    wv_sb[key] = t

            for i in range(JT):
                # A2 chunk jt=i
                ps2 = psA.tile([128, 512], f32, tag="psA", name=f"psA2_{i}")
                for p in range(NP):
                    nc.tensor.matmul(
                        ps2,
                        lhsT=w8slice(wq_sb, p, i * 128),
                        rhs=kview_in[:, 2 * p:2 * p + 2, :],
                        start=(p == 0), stop=(p == NP - 1), perf_mode=DR)
                nc.scalar.activation(
                    kpT8_sb[:, i * SEQ:(i + 1) * SEQ], ps2,
                    Act.Identity, bias=wq_b2T_sb[:, i:i + 1],
                    scale=1.0 / 32.0)
                # A3 tile (tt, js)
                tt, js = divmod(i, HEADS)
                ps3 = psA.tile([128, 512], f32, tag="psA", name=f"psA3_{i}")
                for dt_ in range(DT):
                    nc.tensor.matmul(
                        ps3,
                        lhsT=vT_sb[:, dt_ * SEQ + tt * 128:
                                   dt_ * SEQ + (tt + 1) * 128],
                        rhs=w16slice(wv_sb, dt_, js * 512, 512),
                        start=(dt_ == 0), stop=(dt_ == DT - 1))
                c0 = tt * HD + js * 512
                nc.vector.tensor_add(vp_sb[:, c0:c0 + 512], ps3,
                                     wv_br_sb[:, js * 512:(js + 1) * 512])
                nc.gpsimd.tensor_copy(vp8_sb[:, c0:c0 + 512],
                                      vp_sb[:, c0:c0 + 512])

            # ---- colsum(vp) over all 4096 keys, exact bf16 path ----
            # [1, 512] PE reductions with a ones column as the stationary op.
            pscol = psA.tile([1, 512], f32, tag="psA", name="pscol")
            ci = 0
            for g in range(HEADS):
                for tt in range(TT):
                    nc.tensor.matmul(
                        pscol,
                        lhsT=consts_sb[:, 0:1],
                        rhs=vp_sb[:, tt * HD + g * 512:tt * HD + (g + 1) * 512],
                        start=(ci == 0), stop=(ci == HEADS * TT - 1))
                    ci += 1
            nc.vector.tensor_copy(colrow_sb[0:1, :], pscol)

            # ---- phase B + pipelined out-projection, 4 l-slices ----
            kview = kpT8_sb.rearrange("p (j t) -> p j t", t=SEQ)
            qpview = qpT8_sb.rearrange("p (d hs) -> p d hs", hs=HEADS * S)
            vview = vp8_sb.rearrange("p (t j) -> p t j", j=HD)
            oview = ones8_sb.rearrange("p (two j) -> p two j", two=2)
            onesrow = consts_sb[0:1, 0:512]
            zconst = consts_sb[0:1, 512:640]

            prev_outproj = [None]

            for ls in range(LSLICES):
                h0 = 2 * ls
                psZ = psO.tile([128, 512], f32, tag="psO", name=f"psZ{ls}")
                po = [psO.tile([128, 512], f32, tag="psO", name=f"po{ls}_{i}")
                      for i in range(DT)]
                npair = HEADS * TT // 2  # 16
                # po/psZ accumulation MMs are emitted SKEWP pairs behind the
                # scores+exp of their chunks: at a slice start they block on
                # the previous slice's PSUM drain, and the PE queue is strict
                # FIFO - the skew puts independent scores work ahead of them.
                SKEWP = 3
                pending = []
                ow_tiles = {}

                def emit_pair(pi, g, tt0, rp, psZ=psZ, po=po, npair=npair):
                    rv = rp.rearrange("p (two l) -> p two l", two=2)
                    if pi == 0:
                        # chain heads: +4096 into Z, +colsum into each po[et]
                        nc.tensor.matmul(psZ, lhsT=zconst, rhs=onesrow,
                                         start=True, stop=False)
                        for et in range(DT):
                            nc.tensor.matmul(
                                po[et],
                                lhsT=colrow_sb[0:1, et * 128:(et + 1) * 128],
                                rhs=onesrow, start=True, stop=False)
                    nc.tensor.matmul(psZ, lhsT=oview, rhs=rv,
                                     start=False, stop=(pi == npair - 1),
                                     perf_mode=DR)
                    for et in range(DT):
                        nc.tensor.matmul(
                            po[et],
                            lhsT=vview[:, tt0:tt0 + 2,
                                       g * 512 + et * 128:
                                       g * 512 + (et + 1) * 128],
                            rhs=rv, start=False, stop=(pi == npair - 1),
                            perf_mode=DR)

                rp_cur = [None]
                for g in range(HEADS):
                    for tt in range(TT):
                        ci = g * TT + tt
                        ps = psA.tile([128, 512], f32, tag="psA")
                        # scoresT[(g,tt), (h0..h0+1, s)] = 4*s_raw, fp8 DR
                        for p in range(NP):
                            nc.tensor.matmul(
                                ps,
                                lhsT=kview[:, g * DT + 2 * p:g * DT + 2 * p + 2,
                                           tt * 128:(tt + 1) * 128],
                                rhs=qpview[:, 2 * p:2 * p + 2,
                                           h0 * S:(h0 + 2) * S],
                                start=(p == 0), stop=(p == NP - 1),
                                perf_mode=DR)
                        ext = xp.tile([128, 512], b16, tag="ex")
                        nc.scalar.activation(ext, ps, Act.Exp, bias=0.0,
                                             scale=inv_sqrt_d / 4.0)
                        if tt % 2 == 0:
                            rp_cur[0] = ep.tile([128, 1024], f8, tag="rp",
                                                name=f"rp{ls}_{ci}")
                        # r8 = ex - 1 -> fp8 on DVE: SBUF-to-SBUF gets the
                        # 2x_2p mode (343ns) vs Pool's software rate (806ns)
                        nc.vector.tensor_scalar_sub(
                            rp_cur[0][:, (tt % 2) * 512:(tt % 2) * 512 + 512],
                            ext, 1.0)
                        if tt % 2 == 1:
                            pending.append((ci // 2, g, tt - 1, rp_cur[0]))
                        if ci == 2 and prev_outproj[0] is not None:
                            prev_outproj[0]()
                            prev_outproj[0] = None
                        if ci == 20:
                            # prefetch THIS slice's out-projection weights;
                            # consumed at ci==2 of the next slice (or the
                            # final outproj call)
                            for ct in range(h0 * DT, (h0 + 2) * DT):
                                owt = owp.tile([128, D], b16, tag="ow",
                                               name=f"ow{ct}")
                                nc.sync.dma_start(
                                    out=owt,
                                    in_=owT[ct * 128:(ct + 1) * 128, :])
                                ow_tiles[ct] = owt
                        if len(pending) > SKEWP:
                            emit_pair(*pending.pop(0))
                for args in pending:
                    emit_pair(*args)

                # Z finalization + normalization: Z is fully reduced in psZ
                # (replicated across partitions); normalize po into oT.
                zr = zp.tile([128, 512], f32, tag="zr", name=f"zr{ls}")
                nc.vector.reciprocal(zr, psZ)
                for half in range(2):
                    for et in range(DT):
                        c0 = et * 2048 + ls * 512 + half * 256
                        nc.vector.tensor_mul(
                            oT_sb[:, c0:c0 + 256],
                            po[et][:, half * 256:(half + 1) * 256],
                            zr[:, half * 256:(half + 1) * 256])

                def make_outproj(ls=ls, h0=h0, ow_tiles=ow_tiles):
                    def outproj():
                        # out-projection contribution of this l-slice:
                        # c-tiles ct = h*DT+et for h in (h0, h0+1), bf16
                        for st in range(2):
                            psc = psA.tile([128, 512], f32, tag="psA",
                                           name=f"psc{ls}_{st}")
                            for ci2, ct in enumerate(
                                    range(h0 * DT, (h0 + 2) * DT)):
                                h, et = divmod(ct, DT)
                                nc.tensor.matmul(
                                    psc,
                                    lhsT=oT_sb[:, et * 2048 + h * S + st * 128:
                                               et * 2048 + h * S +
                                               (st + 1) * 128],
                                    rhs=ow_tiles[ct],
                                    start=(ci2 == 0),
                                    stop=(ci2 == 2 * DT - 1))
                            if ls == 0:
                                nc.vector.tensor_add(
                                    fin32[:, st * D:(st + 1) * D],
                                    psc, out_br_sb)
                            else:
                                nc.vector.tensor_add(
                                    fin32[:, st * D:(st + 1) * D],
                                    psc, fin32[:, st * D:(st + 1) * D])
                    return outproj

                prev_outproj[0] = make_outproj()

            prev_outproj[0]()
            for st in range(2):
                nc.sync.dma_start(out=out[st * 128:(st + 1) * 128, :],
                                  in_=fin32[:, st * D:(st + 1) * D])

    nc.compile()
    return nc


def _get_program():
    if "nc" not in _CACHE:
        _CACHE["nc"] = _build_program()
    return _CACHE["nc"]


def _prep_shared(inputs):
    bf = NP_BF16
    e4 = NP_E4
    f32c = np.ascontiguousarray
    consts = np.ones((128, 640), np.float32)
    consts[:, 512:640] = 4096.0
    shared = {
        "wk8": f32c(np.asarray(inputs["wk_w"], np.float32).T * 64).astype(e4),
        "wq8": f32c(np.asarray(inputs["wq_w"], np.float32).T * 64).astype(e4),
        "wvT": f32c(np.asarray(inputs["wv_w"], np.float32).T).astype(bf),
        "owT": f32c(np.asarray(inputs["out_w"], np.float32).T).astype(bf),
        "wk_b2T": f32c(
            (2 * np.asarray(inputs["wk_b"], np.float32)).reshape(JT, 128).T),
        "wq_b2T": f32c(
            (2 * np.asarray(inputs["wq_b"], np.float32)).reshape(JT, 128).T),
        "wv_br": f32c(np.broadcast_to(
            np.asarray(inputs["wv_b"], np.float32)[None, :], (128, HD))),
        "out_br": f32c(np.broadcast_to(
            np.asarray(inputs["out_b"], np.float32)[None, :], (128, D))),
        "ones8": np.ones((128, 256), e4),
        "consts": consts.astype(bf),
    }
    return shared


def _make_in_maps(inputs):
    bf = NP_BF16
    e4 = NP_E4
    shared = _prep_shared(inputs)
    q = np.asarray(inputs["q"], np.float32)
    k = np.asarray(inputs["k"], np.float32)
    v = np.asarray(inputs["v"], np.float32)
    in_maps = []
    for core in range(NCORES):
        b, half = divmod(core, 2)
        m = dict(shared)
        m["qT8"] = np.ascontiguousarray(
            q[b, half * S:(half + 1) * S, :].T).astype(e4)
        m["kT8"] = np.ascontiguousarray(k[b].T).astype(e4)
        m["vT"] = np.ascontiguousarray(v[b].T).astype(bf)
        in_maps.append(m)
    return in_maps


def kernel(**inputs):
    from concourse.bass_utils import run_bass_kernel_spmd

    nc = _get_program()
    in_maps = _make_in_maps(inputs)
    res = run_bass_kernel_spmd(nc, in_maps, core_ids=list(range(NCORES)))
    _CACHE["last_results"] = res
    out = np.empty((BS, SEQ, D), np.float32)
    for core in range(NCORES):
        b, half = divmod(core, 2)
        out[b, half * S:(half + 1) * S, :] = res.results[core]["out"]
    return out


if __name__ == "__main__":
    rng = np.random.default_rng(0)
    fake = {
        "q": rng.standard_normal((BS, SEQ, D)).astype(np.float32),
        "k": rng.standard_normal((BS, SEQ, D)).astype(np.float32),
        "v": rng.standard_normal((BS, SEQ, D)).astype(np.float32),
        "wq_w": (rng.standard_normal((D * HEADS, D)) * 0.02).astype(np.float32),
        "wq_b": (rng.standard_normal((D * HEADS,)) * 0.02).astype(np.float32),
        "wk_w": (rng.standard_normal((D * HEADS, D)) * 0.02).astype(np.float32),
        "wk_b": (rng.standard_normal((D * HEADS,)) * 0.02).astype(np.float32),
        "wv_w": (rng.standard_normal((D * HEADS, D)) * 0.02).astype(np.float32),
        "wv_b": (rng.standard_normal((D * HEADS,)) * 0.02).astype(np.float32),
        "out_w": (rng.standard_normal((D, D * HEADS)) * 0.02).astype(np.float32),
        "out_b": (rng.standard_normal((D,)) * 0.02).astype(np.float32),
    }
    o = kernel(**fake)
    print("kernel ran, out shape", o.shape, "std", o.std())
